# revision 14
# baseline (speedup 1.0000x reference)
"""Adagnn-with-weight GNN message-passing kernel for 8 Trainium2 NeuronCores.

Reference computation (N=100000 nodes, E=3200000 edges, F=256):
    e1  = segment_sum(edge_val[:,None] * x[edge_col], edge_row)   # spmm
    out = (x - e1 * (learnable_diag + 1)) @ weight + bias

Strategy (data-parallel over destination nodes, no collectives):
  - Host: partition edges by destination core (12500 rows each), sort by
    (dest tile of 128 rows, source block of 25000 rows), pad each
    (tile, block) edge list to a multiple of 128 (chunk), take the chunk
    count per cell as the max over cores so all 8 cores run one graph.
  - Device, per dest tile t:
      * dma_gather x rows (bf16) for each source block (int16 idx < 25000)
        into SBUF chunks of 128 edges x 256 feats
      * build one-hot A[e, d] = (iota[d] == dest_local[e]) * val[e] on DVE
      * PE: psum[128 dest, 256] += A_c^T @ Xg_c over all chunks (segment sum)
      * epilogue: e4 = x_own - psum * (diag+1); transpose e4 via PE;
        out = e4 @ W + bias; DMA out (f32).
  - Gather/segment-sum runs in bf16 (values are also bf16-rounded by the
    one-hot A); residual/output path stays f32.
"""

import numpy as np

import concourse.bacc as bacc
import concourse.mybir as mybir
import concourse.tile as tile
from concourse.bass_utils import run_bass_kernel_spmd

FP = mybir.dt.float32
BF = mybir.dt.bfloat16
BF_NP = mybir.dt.np(BF)


class Cfg:
    def __init__(self, n_nodes=100000, n_edges=3200000, f=256, n_cores=8,
                 nb=4, gather_bufs=12, amat_bufs=4, neg_pads=True):
        assert n_nodes % (n_cores * nb) == 0
        self.N = n_nodes
        self.E = n_edges
        self.F = f
        self.NC = n_cores
        self.NB = nb
        self.RPC = n_nodes // n_cores
        self.TILES = (self.RPC + 127) // 128
        self.PAD_ROWS = self.TILES * 128
        self.BLK = n_nodes // nb
        assert self.BLK < (1 << 15)
        self.gather_bufs = gather_bufs
        self.amat_bufs = amat_bufs
        # -1 pad indices skip the pad DMA entirely; only safe once every
        # gather buffer slot has been written at least once (stale-but-finite
        # data times a zero A column is still zero). The first gather_bufs
        # tiles use index-0 pads to warm the slots.
        self.neg_pads = neg_pads


def _preprocess(cfg, edge_row, edge_col, edge_val):
    """Partition + sort + pad the edge list. Returns per-core device arrays
    and the shared static chunk-count table C2[TILES, NB]."""
    edge_row = np.asarray(edge_row).astype(np.int64)
    edge_col = np.asarray(edge_col).astype(np.int64)
    edge_val = np.asarray(edge_val).astype(np.float32)
    NC, TILES, NB, E = cfg.NC, cfg.TILES, cfg.NB, cfg.E

    core = edge_row // cfg.RPC
    dloc = edge_row - core * cfg.RPC
    t = dloc >> 7
    d = (dloc & 127).astype(np.float32)
    b = edge_col // cfg.BLK
    cloc = (edge_col - b * cfg.BLK).astype(np.int16)

    ncell = TILES * NB
    key = core * ncell + t * NB + b
    order = np.lexsort((cloc, key))
    key_s = key[order]

    counts = np.bincount(key, minlength=NC * ncell).reshape(NC, ncell)
    C = np.ceil(counts.max(axis=0) / 128).astype(np.int64)       # [ncell]
    pad_off = np.concatenate([[0], np.cumsum(128 * C)])          # [ncell+1]
    L = int(pad_off[-1])

    # rank of each sorted edge within its (core, cell)
    starts = np.searchsorted(key_s, np.arange(NC * ncell), side="left")
    rank = np.arange(E) - starts[key_s]
    pos = (key_s // ncell) * L + pad_off[key_s % ncell] + rank   # core*L + slot

    if cfg.neg_pads:
        col_pad = np.full(NC * L, -1, dtype=np.int16)
    else:
        col_pad = np.zeros(NC * L, dtype=np.int16)
    dest_pad = np.zeros(NC * L, dtype=np.float32)
    val_pad = np.zeros(NC * L, dtype=np.float32)
    col_pad[pos] = cloc[order]
    dest_pad[pos] = d[order]
    val_pad[pos] = edge_val[order]

    col_pad = col_pad.reshape(NC, L)
    dest_pad = dest_pad.reshape(NC, L)
    val_pad = val_pad.reshape(NC, L)

    # wrapped int16 index layout: element i -> [i % 16, i // 16], x8 replicate
    idx_packed = np.tile(
        col_pad.reshape(NC, L // 16, 16).transpose(0, 2, 1), (1, 8, 1)
    )  # [NC, 128, L//16]

    # per-tile [128, C_t] layouts for dest/val (chunk-major columns), bf16
    C2 = C.reshape(TILES, NB)
    Ct = C2.sum(axis=1)                                          # [TILES]
    CT = int(Ct.sum())
    dest_cols = np.zeros((NC, 128, CT), dtype=BF_NP)
    val_cols = np.zeros((NC, 128, CT), dtype=BF_NP)
    toff = 0
    for tt in range(TILES):
        o0 = int(pad_off[tt * NB])
        n = int(Ct[tt])
        seg = slice(o0, o0 + 128 * n)
        dest_cols[:, :, toff:toff + n] = (
            dest_pad[:, seg].reshape(NC, n, 128).transpose(0, 2, 1))
        val_cols[:, :, toff:toff + n] = (
            val_pad[:, seg].reshape(NC, n, 128).transpose(0, 2, 1))
        toff += n

    return C2, pad_off, idx_packed, dest_cols, val_cols, counts


def _build(cfg, C2, pad_off):
    """Build the (shared) per-core Bass graph given the chunk table."""
    F, NB, TILES = cfg.F, cfg.NB, cfg.TILES
    Ct = C2.sum(axis=1)
    CT = int(Ct.sum())
    L = int(pad_off[-1])
    C_MAXB = int(C2.max())
    C_MAXT = int(Ct.max())

    nc = bacc.Bacc("TRN2", target_bir_lowering=False, debug=False,
                   num_swdge_queues=4)

    xsrc = nc.dram_tensor("xsrc", [cfg.N, F], BF, kind="ExternalInput")
    xown = nc.dram_tensor("xown", [cfg.PAD_ROWS, F], FP, kind="ExternalInput")
    idx_d = nc.dram_tensor("idx", [128, L // 16], mybir.dt.int16,
                           kind="ExternalInput")
    dest_d = nc.dram_tensor("dest", [128, CT], BF, kind="ExternalInput")
    val_d = nc.dram_tensor("val", [128, CT], BF, kind="ExternalInput")
    w_d = nc.dram_tensor("wt", [128, F // 128, F], BF, kind="ExternalInput")
    dscale_d = nc.dram_tensor("dscale", [128, F], FP, kind="ExternalInput")
    bias_d = nc.dram_tensor("bias", [128, F], FP, kind="ExternalInput")
    iota_d = nc.dram_tensor("iota", [128, 128], BF, kind="ExternalInput")
    ident_d = nc.dram_tensor("ident", [128, 128], BF, kind="ExternalInput")
    zeros_d = nc.dram_tensor("zeros", [128, 1], FP, kind="ExternalInput")
    ncalls = int(sum(-(-int(c) // 7) for c in C2.reshape(-1) if c))
    cnt_d = nc.dram_tensor("cnt", [1, max(ncalls, 1)], mybir.dt.int32,
                           kind="ExternalInput")
    out_d = nc.dram_tensor("out", [cfg.PAD_ROWS, F], FP, kind="ExternalOutput")

    KC = F // 128

    with tile.TileContext(nc) as tc:
        with (
            tc.tile_pool(name="const", bufs=1) as cpool,
            tc.tile_pool(name="gather", bufs=cfg.gather_bufs) as gpool,
            tc.tile_pool(name="amat", bufs=cfg.amat_bufs) as apool,
            tc.tile_pool(name="meta", bufs=6) as mpool,
            tc.tile_pool(name="work", bufs=3) as wpool,
            tc.tile_pool(name="pse1", bufs=2, space="PSUM") as e1pool,
            tc.tile_pool(name="pstr", bufs=2, space="PSUM") as trpool,
            tc.tile_pool(name="psout", bufs=2, space="PSUM") as opool,
        ):
            w_t = cpool.tile([128, KC, F], BF)
            dscale_t = cpool.tile([128, F], FP)
            bias_t = cpool.tile([128, F], FP)
            iota_t = cpool.tile([128, 128], BF)
            ident_t = cpool.tile([128, 128], BF)
            zeros_t = cpool.tile([128, 1], FP)
            cnt_t = cpool.tile([1, max(ncalls, 1)], mybir.dt.int32)
            nc.sync.dma_start(w_t[:], w_d[:])
            nc.sync.dma_start(dscale_t[:], dscale_d[:])
            nc.sync.dma_start(bias_t[:], bias_d[:])
            nc.sync.dma_start(iota_t[:], iota_d[:])
            nc.sync.dma_start(ident_t[:], ident_d[:])
            nc.sync.dma_start(zeros_t[:], zeros_d[:])
            nc.sync.dma_start(cnt_t[:], cnt_d[:])
            if cfg.neg_pads:
                # warm every gather slot so skipped pad slots hold finite data
                for _ in range(cfg.gather_bufs):
                    xg_w = gpool.tile([128, C_MAXB, F], BF, tag="xg")
                    nc.vector.memset(xg_w[:], 0.0)

            toff = 0
            qq = 0
            for tt in range(TILES):
                n_t = int(Ct[tt])
                o16 = int(pad_off[tt * NB]) // 16

                idx_t = mpool.tile([128, 8 * n_t], mybir.dt.int16, tag="idx")
                nc.sync.dma_start(idx_t[:], idx_d[:, o16:o16 + 8 * n_t])
                dest_t = mpool.tile([128, n_t], BF, tag="dest")
                nc.sync.dma_start(dest_t[:], dest_d[:, toff:toff + n_t])
                val_t = mpool.tile([128, n_t], BF, tag="val")
                nc.sync.dma_start(val_t[:], val_d[:, toff:toff + n_t])
                xo = wpool.tile([128, F], FP, tag="xo")
                nc.sync.dma_start(xo[:], xown[tt * 128:(tt + 1) * 128, :])

                # gathers, one per non-empty source block
                xgs = []
                coff = 0
                for bb in range(NB):
                    cb = int(C2[tt, bb])
                    if cb == 0:
                        continue
                    xg = gpool.tile([128, C_MAXB, F], BF, tag="xg")
                    for g0 in range(0, cb, 7):
                        gn = min(7, cb - g0)
                        if cfg.neg_pads:
                            nreg = nc.values_load(
                                cnt_t[0:1, qq:qq + 1],
                                engines=[mybir.EngineType.Pool],
                                skip_runtime_bounds_check=True)
                        else:
                            nreg = 128 * gn
                        nc.gpsimd.dma_gather(
                            xg[:, g0:g0 + gn, :],
                            xsrc[bb * cfg.BLK:(bb + 1) * cfg.BLK, :],
                            idx_t[:, 8 * (coff + g0):8 * (coff + g0 + gn)],
                            num_idxs=128 * gn,
                            num_idxs_reg=nreg,
                            elem_size=F,
                            single_packet=True,
                            queue_num=qq % 4,
                        )
                        qq += 1
                    xgs.append((xg, cb))
                    coff += cb

                # one-hot A for the whole tile (merged broadcast ops):
                # A[e, c, :] = (iota == dest[e,c]) * val[e,c]
                a_t = apool.tile([128, C_MAXT, 128], BF, tag="a")
                iota_b = iota_t[:, None, :].broadcast_to((128, n_t, 128))
                dest_b = dest_t[:, :, None].broadcast_to((128, n_t, 128))
                val_b = val_t[:, :, None].broadcast_to((128, n_t, 128))
                nc.vector.tensor_tensor(a_t[:, :n_t, :], iota_b, dest_b,
                                        op=mybir.AluOpType.is_equal)
                nc.vector.tensor_tensor(a_t[:, :n_t, :], a_t[:, :n_t, :],
                                        val_b, op=mybir.AluOpType.mult)

                # segment-sum into PSUM
                e1 = e1pool.tile([128, F], FP, tag="e1")
                cc = 0
                for xg, cb in xgs:
                    for c in range(cb):
                        nc.tensor.matmul(
                            e1[:], a_t[:, cc, :], xg[:, c, :],
                            start=(cc == 0), stop=(cc == n_t - 1),
                        )
                        cc += 1

                # e4 = xo - e1 * dscale   (bf16 result for the projection)
                t0 = wpool.tile([128, F], FP, tag="t0")
                nc.vector.tensor_tensor(t0[:], e1[:], dscale_t[:],
                                        op=mybir.AluOpType.mult)
                e4 = wpool.tile([128, F], BF, tag="e4")
                nc.vector.tensor_tensor(e4[:], xo[:], t0[:],
                                        op=mybir.AluOpType.subtract)

                # transpose e4 (PE), copy to SBUF on ACT
                ps_tr = trpool.tile([128, KC, 128], BF, tag="tr")
                for kc in range(KC):
                    nc.tensor.transpose(ps_tr[:, kc, :],
                                        e4[:, kc * 128:(kc + 1) * 128],
                                        ident_t[:])
                e4T = wpool.tile([128, KC, 128], BF, tag="e4T")
                nc.scalar.activation(e4T[:], ps_tr[:],
                                     mybir.ActivationFunctionType.Identity,
                                     bias=zeros_t[:])

                # out = e4 @ W + bias
                ps_out = opool.tile([128, F], FP, tag="po")
                for kc in range(KC):
                    nc.tensor.matmul(ps_out[:], e4T[:, kc, :], w_t[:, kc, :],
                                     start=(kc == 0), stop=(kc == KC - 1))
                outs = wpool.tile([128, F], FP, tag="outs")
                nc.vector.tensor_tensor(outs[:], ps_out[:], bias_t[:],
                                        op=mybir.AluOpType.add)
                nc.sync.dma_start(out_d[tt * 128:(tt + 1) * 128, :], outs[:])

                toff += n_t

    nc.compile()
    return nc


def _make_in_maps(cfg, x, weight, learnable_diag, bias,
                  idx_packed, dest_cols, val_cols):
    F, NC = cfg.F, cfg.NC
    x16 = x.astype(BF_NP)
    w_host = np.ascontiguousarray(
        weight.reshape(F // 128, 128, F).transpose(1, 0, 2)).astype(BF_NP)
    dscale_host = np.tile((learnable_diag + 1.0)[None, :], (128, 1))
    bias_host = np.tile(bias[None, :], (128, 1))
    iota_host = np.tile(np.arange(128, dtype=np.float32)[None, :],
                        (128, 1)).astype(BF_NP)
    ident_host = np.eye(128, dtype=np.float32).astype(BF_NP)
    zeros_host = np.zeros((128, 1), dtype=np.float32)

    xown_pad = np.zeros((NC, cfg.PAD_ROWS, F), dtype=np.float32)
    xown_pad[:, :cfg.RPC, :] = x.reshape(NC, cfg.RPC, F)

    in_maps = []
    for c in range(NC):
        in_maps.append({
            "xsrc": x16,
            "xown": xown_pad[c],
            "idx": np.ascontiguousarray(idx_packed[c]),
            "dest": np.ascontiguousarray(dest_cols[c]),
            "val": np.ascontiguousarray(val_cols[c]),
            "wt": w_host,
            "dscale": dscale_host,
            "bias": bias_host,
            "iota": iota_host,
            "ident": ident_host,
            "zeros": zeros_host,
        })
    return in_maps


def run(cfg, x, edge_row, edge_col, edge_val, weight, learnable_diag, bias,
        trace_dir=None):
    x = np.ascontiguousarray(np.asarray(x, dtype=np.float32))
    weight = np.asarray(weight, dtype=np.float32)
    learnable_diag = np.asarray(learnable_diag, dtype=np.float32)
    bias = np.asarray(bias, dtype=np.float32)

    C2, pad_off, idx_packed, dest_cols, val_cols, counts = _preprocess(
        cfg, edge_row, edge_col, edge_val)
    nc = _build(cfg, C2, pad_off)
    in_maps = _make_in_maps(cfg, x, weight, learnable_diag, bias,
                            idx_packed, dest_cols, val_cols)
    # per-core per-sub-call true index counts (for pad-skipping);
    # sub-calls cover 7-chunk windows of each (tile, block) cell
    flatC = C2.reshape(-1)
    for c in range(cfg.NC):
        sub = []
        for cell, cb in enumerate(flatC):
            cb = int(cb)
            if cb == 0:
                continue
            n = int(counts[c][cell])
            for g0 in range(0, cb, 7):
                gn = min(7, cb - g0)
                sub.append(int(np.clip(n - 128 * g0, 0, 128 * gn)))
        cc = np.asarray(sub, dtype=np.int32)
        in_maps[c]["cnt"] = cc.reshape(1, -1) if len(cc) else np.zeros(
            (1, 1), np.int32)

    kwargs = {}
    if trace_dir:
        kwargs = dict(trace=True, tmpdir=trace_dir)
    res = run_bass_kernel_spmd(nc, in_maps, core_ids=list(range(cfg.NC)),
                               **kwargs)
    out = np.empty((cfg.N, cfg.F), dtype=np.float32)
    for c in range(cfg.NC):
        out[c * cfg.RPC:(c + 1) * cfg.RPC] = res.results[c]["out"][:cfg.RPC]
    return out, res


def kernel(x, edge_row, edge_col, edge_val, weight, learnable_diag, bias,
           _want_trace=None):
    cfg = Cfg()
    out, res = run(cfg, x, edge_row, edge_col, edge_val, weight,
                   learnable_diag, bias, trace_dir=_want_trace)
    kernel._last_results = res
    return out


# revision 15
# speedup vs baseline: 1.5620x; 1.5620x over previous
"""Adagnn-with-weight GNN message-passing kernel for 8 Trainium2 NeuronCores.

Reference computation (N=100000 nodes, E=3200000 edges, F=256):
    e1  = segment_sum(edge_val[:,None] * x[edge_col], edge_row)   # spmm
    out = (x - e1 * (learnable_diag + 1)) @ weight + bias

Strategy (data-parallel over destination nodes, no collectives):
  - Host: partition edges by destination core (12500 rows each), sort by
    (dest tile of 128 rows, source block of 25000 rows), pad each
    (tile, block) edge list to a multiple of 128 (chunk), take the chunk
    count per cell as the max over cores so all 8 cores run one graph.
  - Device, per dest tile t:
      * dma_gather x rows (bf16) for each source block (int16 idx < 25000)
        into SBUF chunks of 128 edges x 256 feats
      * build one-hot A[e, d] = (iota[d] == dest_local[e]) * val[e] on DVE
      * PE: psum[128 dest, 256] += A_c^T @ Xg_c over all chunks (segment sum)
      * epilogue: e4 = x_own - psum * (diag+1); transpose e4 via PE;
        out = e4 @ W + bias; DMA out (f32).
  - Gather/segment-sum runs in bf16 (values are also bf16-rounded by the
    one-hot A); residual/output path stays f32.
"""

import numpy as np

import concourse.bacc as bacc
import concourse.mybir as mybir
import concourse.tile as tile
from concourse.bass_utils import run_bass_kernel_spmd

FP = mybir.dt.float32
BF = mybir.dt.bfloat16
BF_NP = mybir.dt.np(BF)


class Cfg:
    def __init__(self, n_nodes=100000, n_edges=3200000, f=256, n_cores=8,
                 nb=4, gather_bufs=12, amat_bufs=4, neg_pads=True):
        assert n_nodes % (n_cores * nb) == 0
        self.N = n_nodes
        self.E = n_edges
        self.F = f
        self.NC = n_cores
        self.NB = nb
        self.RPC = n_nodes // n_cores
        self.TILES = (self.RPC + 127) // 128
        self.PAD_ROWS = self.TILES * 128
        self.BLK = n_nodes // nb
        assert self.BLK < (1 << 15)
        self.gather_bufs = gather_bufs
        self.amat_bufs = amat_bufs
        # -1 pad indices skip the pad DMA entirely; only safe once every
        # gather buffer slot has been written at least once (stale-but-finite
        # data times a zero A column is still zero). The first gather_bufs
        # tiles use index-0 pads to warm the slots.
        self.neg_pads = neg_pads


def _preprocess(cfg, edge_row, edge_col, edge_val):
    """Partition + sort + pad the edge list. Returns per-core device arrays
    and the shared static chunk-count table C2[TILES, NB]."""
    edge_row = np.asarray(edge_row).astype(np.int64)
    edge_col = np.asarray(edge_col).astype(np.int64)
    edge_val = np.asarray(edge_val).astype(np.float32)
    NC, TILES, NB, E = cfg.NC, cfg.TILES, cfg.NB, cfg.E

    core = edge_row // cfg.RPC
    dloc = edge_row - core * cfg.RPC
    t = dloc >> 7
    d = (dloc & 127).astype(np.float32)
    b = edge_col // cfg.BLK
    cloc = (edge_col - b * cfg.BLK).astype(np.int16)

    ncell = TILES * NB
    key = core * ncell + t * NB + b
    order = np.lexsort((cloc, key))
    key_s = key[order]

    counts = np.bincount(key, minlength=NC * ncell).reshape(NC, ncell)
    C = np.ceil(counts.max(axis=0) / 128).astype(np.int64)       # [ncell]
    pad_off = np.concatenate([[0], np.cumsum(128 * C)])          # [ncell+1]
    L = int(pad_off[-1])

    # rank of each sorted edge within its (core, cell)
    starts = np.searchsorted(key_s, np.arange(NC * ncell), side="left")
    rank = np.arange(E) - starts[key_s]
    pos = (key_s // ncell) * L + pad_off[key_s % ncell] + rank   # core*L + slot

    if cfg.neg_pads:
        col_pad = np.full(NC * L, -1, dtype=np.int16)
    else:
        col_pad = np.zeros(NC * L, dtype=np.int16)
    dest_pad = np.zeros(NC * L, dtype=np.float32)
    val_pad = np.zeros(NC * L, dtype=np.float32)
    col_pad[pos] = cloc[order]
    dest_pad[pos] = d[order]
    val_pad[pos] = edge_val[order]

    col_pad = col_pad.reshape(NC, L)
    dest_pad = dest_pad.reshape(NC, L)
    val_pad = val_pad.reshape(NC, L)

    # wrapped int16 index layout: element i -> [i % 16, i // 16], x8 replicate
    idx_packed = np.tile(
        col_pad.reshape(NC, L // 16, 16).transpose(0, 2, 1), (1, 8, 1)
    )  # [NC, 128, L//16]

    # per-tile [128, C_t] layouts for dest/val (chunk-major columns), bf16
    C2 = C.reshape(TILES, NB)
    Ct = C2.sum(axis=1)                                          # [TILES]
    CT = int(Ct.sum())
    dest_cols = np.zeros((NC, 128, CT), dtype=BF_NP)
    val_cols = np.zeros((NC, 128, CT), dtype=BF_NP)
    toff = 0
    for tt in range(TILES):
        o0 = int(pad_off[tt * NB])
        n = int(Ct[tt])
        seg = slice(o0, o0 + 128 * n)
        dest_cols[:, :, toff:toff + n] = (
            dest_pad[:, seg].reshape(NC, n, 128).transpose(0, 2, 1))
        val_cols[:, :, toff:toff + n] = (
            val_pad[:, seg].reshape(NC, n, 128).transpose(0, 2, 1))
        toff += n

    return C2, pad_off, idx_packed, dest_cols, val_cols, counts


def _build(cfg, C2, pad_off):
    """Build the (shared) per-core Bass graph given the chunk table."""
    F, NB, TILES = cfg.F, cfg.NB, cfg.TILES
    Ct = C2.sum(axis=1)
    CT = int(Ct.sum())
    L = int(pad_off[-1])
    C_MAXB = int(C2.max())
    C_MAXT = int(Ct.max())

    nc = bacc.Bacc("TRN2", target_bir_lowering=False, debug=False,
                   num_swdge_queues=4)

    xsrc = nc.dram_tensor("xsrc", [cfg.N, F], BF, kind="ExternalInput")
    xown = nc.dram_tensor("xown", [cfg.PAD_ROWS, F], FP, kind="ExternalInput")
    idx_d = nc.dram_tensor("idx", [128, L // 16], mybir.dt.int16,
                           kind="ExternalInput")
    dest_d = nc.dram_tensor("dest", [128, CT], BF, kind="ExternalInput")
    val_d = nc.dram_tensor("val", [128, CT], BF, kind="ExternalInput")
    w_d = nc.dram_tensor("wt", [128, F // 128, F], BF, kind="ExternalInput")
    dscale_d = nc.dram_tensor("dscale", [128, F], FP, kind="ExternalInput")
    bias_d = nc.dram_tensor("bias", [128, F], FP, kind="ExternalInput")
    iota_d = nc.dram_tensor("iota", [128, 128], BF, kind="ExternalInput")
    ident_d = nc.dram_tensor("ident", [128, 128], BF, kind="ExternalInput")
    zeros_d = nc.dram_tensor("zeros", [128, 1], FP, kind="ExternalInput")
    ncalls = int((C2 > 0).sum())
    cnt_d = nc.dram_tensor("cnt", [1, max(ncalls, 1)], mybir.dt.int32,
                           kind="ExternalInput")
    out_d = nc.dram_tensor("out", [cfg.PAD_ROWS, F], FP, kind="ExternalOutput")

    KC = F // 128

    with tile.TileContext(nc) as tc:
        with (
            tc.tile_pool(name="const", bufs=1) as cpool,
            tc.tile_pool(name="gather", bufs=cfg.gather_bufs) as gpool,
            tc.tile_pool(name="amat", bufs=cfg.amat_bufs) as apool,
            tc.tile_pool(name="meta", bufs=6) as mpool,
            tc.tile_pool(name="work", bufs=3) as wpool,
            tc.tile_pool(name="pse1", bufs=2, space="PSUM") as e1pool,
            tc.tile_pool(name="pstr", bufs=2, space="PSUM") as trpool,
            tc.tile_pool(name="psout", bufs=2, space="PSUM") as opool,
        ):
            w_t = cpool.tile([128, KC, F], BF)
            dscale_t = cpool.tile([128, F], FP)
            bias_t = cpool.tile([128, F], FP)
            iota_t = cpool.tile([128, 128], BF)
            ident_t = cpool.tile([128, 128], BF)
            zeros_t = cpool.tile([128, 1], FP)
            cnt_t = cpool.tile([1, max(ncalls, 1)], mybir.dt.int32)
            nc.sync.dma_start(w_t[:], w_d[:])
            nc.sync.dma_start(dscale_t[:], dscale_d[:])
            nc.sync.dma_start(bias_t[:], bias_d[:])
            nc.sync.dma_start(iota_t[:], iota_d[:])
            nc.sync.dma_start(ident_t[:], ident_d[:])
            nc.sync.dma_start(zeros_t[:], zeros_d[:])
            nc.sync.dma_start(cnt_t[:], cnt_d[:])
            if cfg.neg_pads:
                # warm every gather slot so skipped pad slots hold finite data
                for _ in range(cfg.gather_bufs):
                    xg_w = gpool.tile([128, C_MAXB, F], BF, tag="xg")
                    nc.vector.memset(xg_w[:], 0.0)

            toff = 0
            qq = 0
            for tt in range(TILES):
                n_t = int(Ct[tt])
                o16 = int(pad_off[tt * NB]) // 16

                idx_t = mpool.tile([128, 8 * n_t], mybir.dt.int16, tag="idx")
                nc.sync.dma_start(idx_t[:], idx_d[:, o16:o16 + 8 * n_t])
                dest_t = mpool.tile([128, n_t], BF, tag="dest")
                nc.sync.dma_start(dest_t[:], dest_d[:, toff:toff + n_t])
                val_t = mpool.tile([128, n_t], BF, tag="val")
                nc.sync.dma_start(val_t[:], val_d[:, toff:toff + n_t])
                xo = wpool.tile([128, F], FP, tag="xo")
                nc.sync.dma_start(xo[:], xown[tt * 128:(tt + 1) * 128, :])

                # gathers, one per non-empty source block
                xgs = []
                coff = 0
                for bb in range(NB):
                    cb = int(C2[tt, bb])
                    if cb == 0:
                        continue
                    xg = gpool.tile([128, C_MAXB, F], BF, tag="xg")
                    if cfg.neg_pads:
                        nreg = nc.values_load(
                            cnt_t[0:1, qq:qq + 1],
                            engines=[mybir.EngineType.Pool],
                            skip_runtime_bounds_check=True)
                    else:
                        nreg = 128 * cb
                    nc.gpsimd.dma_gather(
                        xg[:, :cb, :],
                        xsrc[bb * cfg.BLK:(bb + 1) * cfg.BLK, :],
                        idx_t[:, 8 * coff:8 * (coff + cb)],
                        num_idxs=128 * cb,
                        num_idxs_reg=nreg,
                        elem_size=F,
                        single_packet=False,
                        queue_num=qq % 4,
                    )
                    qq += 1
                    xgs.append((xg, cb))
                    coff += cb

                # one-hot A for the whole tile (merged broadcast ops):
                # A[e, c, :] = (iota == dest[e,c]) * val[e,c]
                a_t = apool.tile([128, C_MAXT, 128], BF, tag="a")
                iota_b = iota_t[:, None, :].broadcast_to((128, n_t, 128))
                dest_b = dest_t[:, :, None].broadcast_to((128, n_t, 128))
                val_b = val_t[:, :, None].broadcast_to((128, n_t, 128))
                nc.vector.tensor_tensor(a_t[:, :n_t, :], iota_b, dest_b,
                                        op=mybir.AluOpType.is_equal)
                nc.vector.tensor_tensor(a_t[:, :n_t, :], a_t[:, :n_t, :],
                                        val_b, op=mybir.AluOpType.mult)

                # segment-sum into PSUM
                e1 = e1pool.tile([128, F], FP, tag="e1")
                cc = 0
                for xg, cb in xgs:
                    for c in range(cb):
                        nc.tensor.matmul(
                            e1[:], a_t[:, cc, :], xg[:, c, :],
                            start=(cc == 0), stop=(cc == n_t - 1),
                        )
                        cc += 1

                # e4 = xo - e1 * dscale   (bf16 result for the projection)
                t0 = wpool.tile([128, F], FP, tag="t0")
                nc.vector.tensor_tensor(t0[:], e1[:], dscale_t[:],
                                        op=mybir.AluOpType.mult)
                e4 = wpool.tile([128, F], BF, tag="e4")
                nc.vector.tensor_tensor(e4[:], xo[:], t0[:],
                                        op=mybir.AluOpType.subtract)

                # transpose e4 (PE), copy to SBUF on ACT
                ps_tr = trpool.tile([128, KC, 128], BF, tag="tr")
                for kc in range(KC):
                    nc.tensor.transpose(ps_tr[:, kc, :],
                                        e4[:, kc * 128:(kc + 1) * 128],
                                        ident_t[:])
                e4T = wpool.tile([128, KC, 128], BF, tag="e4T")
                nc.scalar.activation(e4T[:], ps_tr[:],
                                     mybir.ActivationFunctionType.Identity,
                                     bias=zeros_t[:])

                # out = e4 @ W + bias
                ps_out = opool.tile([128, F], FP, tag="po")
                for kc in range(KC):
                    nc.tensor.matmul(ps_out[:], e4T[:, kc, :], w_t[:, kc, :],
                                     start=(kc == 0), stop=(kc == KC - 1))
                outs = wpool.tile([128, F], FP, tag="outs")
                nc.vector.tensor_tensor(outs[:], ps_out[:], bias_t[:],
                                        op=mybir.AluOpType.add)
                nc.sync.dma_start(out_d[tt * 128:(tt + 1) * 128, :], outs[:])

                toff += n_t

    nc.compile()
    return nc


def _make_in_maps(cfg, x, weight, learnable_diag, bias,
                  idx_packed, dest_cols, val_cols):
    F, NC = cfg.F, cfg.NC
    x16 = x.astype(BF_NP)
    w_host = np.ascontiguousarray(
        weight.reshape(F // 128, 128, F).transpose(1, 0, 2)).astype(BF_NP)
    dscale_host = np.tile((learnable_diag + 1.0)[None, :], (128, 1))
    bias_host = np.tile(bias[None, :], (128, 1))
    iota_host = np.tile(np.arange(128, dtype=np.float32)[None, :],
                        (128, 1)).astype(BF_NP)
    ident_host = np.eye(128, dtype=np.float32).astype(BF_NP)
    zeros_host = np.zeros((128, 1), dtype=np.float32)

    xown_pad = np.zeros((NC, cfg.PAD_ROWS, F), dtype=np.float32)
    xown_pad[:, :cfg.RPC, :] = x.reshape(NC, cfg.RPC, F)

    in_maps = []
    for c in range(NC):
        in_maps.append({
            "xsrc": x16,
            "xown": xown_pad[c],
            "idx": np.ascontiguousarray(idx_packed[c]),
            "dest": np.ascontiguousarray(dest_cols[c]),
            "val": np.ascontiguousarray(val_cols[c]),
            "wt": w_host,
            "dscale": dscale_host,
            "bias": bias_host,
            "iota": iota_host,
            "ident": ident_host,
            "zeros": zeros_host,
        })
    return in_maps


def run(cfg, x, edge_row, edge_col, edge_val, weight, learnable_diag, bias,
        trace_dir=None):
    x = np.ascontiguousarray(np.asarray(x, dtype=np.float32))
    weight = np.asarray(weight, dtype=np.float32)
    learnable_diag = np.asarray(learnable_diag, dtype=np.float32)
    bias = np.asarray(bias, dtype=np.float32)

    C2, pad_off, idx_packed, dest_cols, val_cols, counts = _preprocess(
        cfg, edge_row, edge_col, edge_val)
    nc = _build(cfg, C2, pad_off)
    in_maps = _make_in_maps(cfg, x, weight, learnable_diag, bias,
                            idx_packed, dest_cols, val_cols)
    # per-core per-gather-call true index counts (for pad-skipping)
    nonempty = (C2 > 0).reshape(-1)
    for c in range(cfg.NC):
        cc = counts[c][nonempty].astype(np.int32)
        in_maps[c]["cnt"] = cc.reshape(1, -1) if len(cc) else np.zeros(
            (1, 1), np.int32)

    kwargs = {}
    if trace_dir:
        kwargs = dict(trace=True, tmpdir=trace_dir)
    res = run_bass_kernel_spmd(nc, in_maps, core_ids=list(range(cfg.NC)),
                               **kwargs)
    out = np.empty((cfg.N, cfg.F), dtype=np.float32)
    for c in range(cfg.NC):
        out[c * cfg.RPC:(c + 1) * cfg.RPC] = res.results[c]["out"][:cfg.RPC]
    return out, res


def kernel(x, edge_row, edge_col, edge_val, weight, learnable_diag, bias,
           _want_trace=None):
    cfg = Cfg()
    out, res = run(cfg, x, edge_row, edge_col, edge_val, weight,
                   learnable_diag, bias, trace_dir=_want_trace)
    kernel._last_results = res
    return out


# revision 16
# speedup vs baseline: 1.5726x; 1.0068x over previous
"""Adagnn-with-weight GNN message-passing kernel for 8 Trainium2 NeuronCores.

Reference computation (N=100000 nodes, E=3200000 edges, F=256):
    e1  = segment_sum(edge_val[:,None] * x[edge_col], edge_row)   # spmm
    out = (x - e1 * (learnable_diag + 1)) @ weight + bias

Strategy (data-parallel over destination nodes, no collectives):
  - Host: partition edges by destination core (12500 rows each), sort by
    (dest tile of 128 rows, source block of 25000 rows), pad each
    (tile, block) edge list to a multiple of 128 (chunk), take the chunk
    count per cell as the max over cores so all 8 cores run one graph.
  - Device, per dest tile t:
      * dma_gather x rows (bf16) for each source block (int16 idx < 25000)
        into SBUF chunks of 128 edges x 256 feats
      * build one-hot A[e, d] = (iota[d] == dest_local[e]) * val[e] on DVE
      * PE: psum[128 dest, 256] += A_c^T @ Xg_c over all chunks (segment sum)
      * epilogue: e4 = x_own - psum * (diag+1); transpose e4 via PE;
        out = e4 @ W + bias; DMA out (f32).
  - Gather/segment-sum runs in bf16 (values are also bf16-rounded by the
    one-hot A); residual/output path stays f32.
"""

import numpy as np

import concourse.bacc as bacc
import concourse.mybir as mybir
import concourse.tile as tile
from concourse.bass_utils import run_bass_kernel_spmd

FP = mybir.dt.float32
BF = mybir.dt.bfloat16
BF_NP = mybir.dt.np(BF)


class Cfg:
    def __init__(self, n_nodes=100000, n_edges=3200000, f=256, n_cores=8,
                 nb=4, gather_bufs=12, amat_bufs=4, neg_pads=True):
        assert n_nodes % (n_cores * nb) == 0
        self.N = n_nodes
        self.E = n_edges
        self.F = f
        self.NC = n_cores
        self.NB = nb
        self.RPC = n_nodes // n_cores
        self.TILES = (self.RPC + 127) // 128
        self.PAD_ROWS = self.TILES * 128
        self.BLK = n_nodes // nb
        assert self.BLK < (1 << 15)
        self.gather_bufs = gather_bufs
        self.amat_bufs = amat_bufs
        # -1 pad indices skip the pad DMA entirely; only safe once every
        # gather buffer slot has been written at least once (stale-but-finite
        # data times a zero A column is still zero). The first gather_bufs
        # tiles use index-0 pads to warm the slots.
        self.neg_pads = neg_pads


def _preprocess(cfg, edge_row, edge_col, edge_val):
    """Partition + sort + pad the edge list. Returns per-core device arrays
    and the shared static chunk-count table C2[TILES, NB]."""
    edge_row = np.asarray(edge_row).astype(np.int64)
    edge_col = np.asarray(edge_col).astype(np.int64)
    edge_val = np.asarray(edge_val).astype(np.float32)
    NC, TILES, NB, E = cfg.NC, cfg.TILES, cfg.NB, cfg.E

    core = edge_row // cfg.RPC
    dloc = edge_row - core * cfg.RPC
    t = dloc >> 7
    d = (dloc & 127).astype(np.float32)
    b = edge_col // cfg.BLK
    cloc = (edge_col - b * cfg.BLK).astype(np.int16)

    ncell = TILES * NB
    key = core * ncell + t * NB + b
    order = np.lexsort((cloc, key))
    key_s = key[order]

    counts = np.bincount(key, minlength=NC * ncell).reshape(NC, ncell)
    C = np.ceil(counts.max(axis=0) / 128).astype(np.int64)       # [ncell]
    pad_off = np.concatenate([[0], np.cumsum(128 * C)])          # [ncell+1]
    L = int(pad_off[-1])

    # rank of each sorted edge within its (core, cell)
    starts = np.searchsorted(key_s, np.arange(NC * ncell), side="left")
    rank = np.arange(E) - starts[key_s]
    pos = (key_s // ncell) * L + pad_off[key_s % ncell] + rank   # core*L + slot

    if cfg.neg_pads:
        col_pad = np.full(NC * L, -1, dtype=np.int16)
    else:
        col_pad = np.zeros(NC * L, dtype=np.int16)
    dest_pad = np.zeros(NC * L, dtype=np.float32)
    val_pad = np.zeros(NC * L, dtype=np.float32)
    col_pad[pos] = cloc[order]
    dest_pad[pos] = d[order]
    val_pad[pos] = edge_val[order]

    col_pad = col_pad.reshape(NC, L)
    dest_pad = dest_pad.reshape(NC, L)
    val_pad = val_pad.reshape(NC, L)

    # wrapped int16 index layout: element i -> [i % 16, i // 16], x8 replicate
    idx_packed = np.tile(
        col_pad.reshape(NC, L // 16, 16).transpose(0, 2, 1), (1, 8, 1)
    )  # [NC, 128, L//16]

    # per-tile [128, C_t] layouts for dest/val (chunk-major columns), bf16
    C2 = C.reshape(TILES, NB)
    Ct = C2.sum(axis=1)                                          # [TILES]
    CT = int(Ct.sum())
    dest_cols = np.zeros((NC, 128, CT), dtype=BF_NP)
    val_cols = np.zeros((NC, 128, CT), dtype=BF_NP)
    toff = 0
    for tt in range(TILES):
        o0 = int(pad_off[tt * NB])
        n = int(Ct[tt])
        seg = slice(o0, o0 + 128 * n)
        dest_cols[:, :, toff:toff + n] = (
            dest_pad[:, seg].reshape(NC, n, 128).transpose(0, 2, 1))
        val_cols[:, :, toff:toff + n] = (
            val_pad[:, seg].reshape(NC, n, 128).transpose(0, 2, 1))
        toff += n

    return C2, pad_off, idx_packed, dest_cols, val_cols, counts


def _build(cfg, C2, pad_off):
    """Build the (shared) per-core Bass graph given the chunk table."""
    F, NB, TILES = cfg.F, cfg.NB, cfg.TILES
    Ct = C2.sum(axis=1)
    CT = int(Ct.sum())
    L = int(pad_off[-1])
    C_MAXB = int(C2.max())
    C_MAXT = int(Ct.max())

    nc = bacc.Bacc("TRN2", target_bir_lowering=False, debug=False,
                   num_swdge_queues=4)

    xsrc = nc.dram_tensor("xsrc", [cfg.N, F], BF, kind="ExternalInput")
    xown = nc.dram_tensor("xown", [cfg.PAD_ROWS, F], FP, kind="ExternalInput")
    idx_d = nc.dram_tensor("idx", [128, L // 16], mybir.dt.int16,
                           kind="ExternalInput")
    dest_d = nc.dram_tensor("dest", [128, CT], BF, kind="ExternalInput")
    val_d = nc.dram_tensor("val", [128, CT], BF, kind="ExternalInput")
    w_d = nc.dram_tensor("wt", [128, F // 128, F], BF, kind="ExternalInput")
    dscale_d = nc.dram_tensor("dscale", [128, F], FP, kind="ExternalInput")
    bias_d = nc.dram_tensor("bias", [128, F], FP, kind="ExternalInput")
    iota_d = nc.dram_tensor("iota", [128, 128], BF, kind="ExternalInput")
    ident_d = nc.dram_tensor("ident", [128, 128], BF, kind="ExternalInput")
    zeros_d = nc.dram_tensor("zeros", [128, 1], FP, kind="ExternalInput")
    ncalls = int((C2 > 0).sum())
    cnt_d = nc.dram_tensor("cnt", [1, max(ncalls, 1)], mybir.dt.int32,
                           kind="ExternalInput")
    out_d = nc.dram_tensor("out", [cfg.PAD_ROWS, F], FP, kind="ExternalOutput")

    KC = F // 128

    with tile.TileContext(nc) as tc:
        with (
            tc.tile_pool(name="const", bufs=1) as cpool,
            tc.tile_pool(name="gather", bufs=cfg.gather_bufs) as gpool,
            tc.tile_pool(name="amat", bufs=cfg.amat_bufs) as apool,
            tc.tile_pool(name="meta", bufs=6) as mpool,
            tc.tile_pool(name="work", bufs=3) as wpool,
            tc.tile_pool(name="pse1", bufs=2, space="PSUM") as e1pool,
            tc.tile_pool(name="pstr", bufs=2, space="PSUM") as trpool,
            tc.tile_pool(name="psout", bufs=2, space="PSUM") as opool,
        ):
            w_t = cpool.tile([128, KC, F], BF)
            dscale_t = cpool.tile([128, F], FP)
            bias_t = cpool.tile([128, F], FP)
            iota_t = cpool.tile([128, 128], BF)
            ident_t = cpool.tile([128, 128], BF)
            zeros_t = cpool.tile([128, 1], FP)
            cnt_t = cpool.tile([1, max(ncalls, 1)], mybir.dt.int32)
            nc.sync.dma_start(w_t[:], w_d[:])
            nc.sync.dma_start(dscale_t[:], dscale_d[:])
            nc.sync.dma_start(bias_t[:], bias_d[:])
            nc.sync.dma_start(iota_t[:], iota_d[:])
            nc.sync.dma_start(ident_t[:], ident_d[:])
            nc.sync.dma_start(zeros_t[:], zeros_d[:])
            nc.sync.dma_start(cnt_t[:], cnt_d[:])
            if cfg.neg_pads:
                # warm every gather slot so skipped pad slots hold finite data
                for _ in range(cfg.gather_bufs):
                    xg_w = gpool.tile([128, C_MAXB, F], BF, tag="xg")
                    nc.vector.memset(xg_w[:], 0.0)

            toff = 0
            qq = 0
            for tt in range(TILES):
                n_t = int(Ct[tt])
                o16 = int(pad_off[tt * NB]) // 16

                idx_t = mpool.tile([128, 8 * n_t], mybir.dt.int16, tag="idx")
                nc.sync.dma_start(idx_t[:], idx_d[:, o16:o16 + 8 * n_t])
                dest_t = mpool.tile([128, n_t], BF, tag="dest")
                nc.sync.dma_start(dest_t[:], dest_d[:, toff:toff + n_t])
                val_t = mpool.tile([128, n_t], BF, tag="val")
                nc.sync.dma_start(val_t[:], val_d[:, toff:toff + n_t])
                xo = wpool.tile([128, F], FP, tag="xo")
                nc.sync.dma_start(xo[:], xown[tt * 128:(tt + 1) * 128, :])

                # one-hot A for the whole tile (merged broadcast ops):
                # A[e, c, :] = (iota == dest[e,c]) * val[e,c]
                a_t = apool.tile([128, C_MAXT, 128], BF, tag="a")
                iota_b = iota_t[:, None, :].broadcast_to((128, n_t, 128))
                dest_b = dest_t[:, :, None].broadcast_to((128, n_t, 128))
                val_b = val_t[:, :, None].broadcast_to((128, n_t, 128))
                nc.vector.tensor_tensor(a_t[:, :n_t, :], iota_b, dest_b,
                                        op=mybir.AluOpType.is_equal)
                nc.vector.tensor_tensor(a_t[:, :n_t, :], a_t[:, :n_t, :],
                                        val_b, op=mybir.AluOpType.mult)

                # gathers, one per non-empty source block
                xgs = []
                coff = 0
                for bb in range(NB):
                    cb = int(C2[tt, bb])
                    if cb == 0:
                        continue
                    xg = gpool.tile([128, C_MAXB, F], BF, tag="xg")
                    if cfg.neg_pads:
                        nreg = nc.values_load(
                            cnt_t[0:1, qq:qq + 1],
                            engines=[mybir.EngineType.Pool],
                            skip_runtime_bounds_check=True)
                    else:
                        nreg = 128 * cb
                    nc.gpsimd.dma_gather(
                        xg[:, :cb, :],
                        xsrc[bb * cfg.BLK:(bb + 1) * cfg.BLK, :],
                        idx_t[:, 8 * coff:8 * (coff + cb)],
                        num_idxs=128 * cb,
                        num_idxs_reg=nreg,
                        elem_size=F,
                        single_packet=False,
                        queue_num=qq % 4,
                    )
                    qq += 1
                    xgs.append((xg, cb))
                    coff += cb

                # segment-sum into PSUM
                e1 = e1pool.tile([128, F], FP, tag="e1")
                cc = 0
                for xg, cb in xgs:
                    for c in range(cb):
                        nc.tensor.matmul(
                            e1[:], a_t[:, cc, :], xg[:, c, :],
                            start=(cc == 0), stop=(cc == n_t - 1),
                        )
                        cc += 1

                # e4 = xo - e1 * dscale   (bf16 result for the projection)
                t0 = wpool.tile([128, F], FP, tag="t0")
                nc.vector.tensor_tensor(t0[:], e1[:], dscale_t[:],
                                        op=mybir.AluOpType.mult)
                e4 = wpool.tile([128, F], BF, tag="e4")
                nc.vector.tensor_tensor(e4[:], xo[:], t0[:],
                                        op=mybir.AluOpType.subtract)

                # transpose e4 (PE), copy to SBUF on ACT
                ps_tr = trpool.tile([128, KC, 128], BF, tag="tr")
                for kc in range(KC):
                    nc.tensor.transpose(ps_tr[:, kc, :],
                                        e4[:, kc * 128:(kc + 1) * 128],
                                        ident_t[:])
                e4T = wpool.tile([128, KC, 128], BF, tag="e4T")
                nc.scalar.activation(e4T[:], ps_tr[:],
                                     mybir.ActivationFunctionType.Identity,
                                     bias=zeros_t[:])

                # out = e4 @ W + bias
                ps_out = opool.tile([128, F], FP, tag="po")
                for kc in range(KC):
                    nc.tensor.matmul(ps_out[:], e4T[:, kc, :], w_t[:, kc, :],
                                     start=(kc == 0), stop=(kc == KC - 1))
                outs = wpool.tile([128, F], FP, tag="outs")
                nc.vector.tensor_tensor(outs[:], ps_out[:], bias_t[:],
                                        op=mybir.AluOpType.add)
                nc.sync.dma_start(out_d[tt * 128:(tt + 1) * 128, :], outs[:])

                toff += n_t

    nc.compile()
    return nc


def _make_in_maps(cfg, x, weight, learnable_diag, bias,
                  idx_packed, dest_cols, val_cols):
    F, NC = cfg.F, cfg.NC
    x16 = x.astype(BF_NP)
    w_host = np.ascontiguousarray(
        weight.reshape(F // 128, 128, F).transpose(1, 0, 2)).astype(BF_NP)
    dscale_host = np.tile((learnable_diag + 1.0)[None, :], (128, 1))
    bias_host = np.tile(bias[None, :], (128, 1))
    iota_host = np.tile(np.arange(128, dtype=np.float32)[None, :],
                        (128, 1)).astype(BF_NP)
    ident_host = np.eye(128, dtype=np.float32).astype(BF_NP)
    zeros_host = np.zeros((128, 1), dtype=np.float32)

    xown_pad = np.zeros((NC, cfg.PAD_ROWS, F), dtype=np.float32)
    xown_pad[:, :cfg.RPC, :] = x.reshape(NC, cfg.RPC, F)

    in_maps = []
    for c in range(NC):
        in_maps.append({
            "xsrc": x16,
            "xown": xown_pad[c],
            "idx": np.ascontiguousarray(idx_packed[c]),
            "dest": np.ascontiguousarray(dest_cols[c]),
            "val": np.ascontiguousarray(val_cols[c]),
            "wt": w_host,
            "dscale": dscale_host,
            "bias": bias_host,
            "iota": iota_host,
            "ident": ident_host,
            "zeros": zeros_host,
        })
    return in_maps


def run(cfg, x, edge_row, edge_col, edge_val, weight, learnable_diag, bias,
        trace_dir=None):
    x = np.ascontiguousarray(np.asarray(x, dtype=np.float32))
    weight = np.asarray(weight, dtype=np.float32)
    learnable_diag = np.asarray(learnable_diag, dtype=np.float32)
    bias = np.asarray(bias, dtype=np.float32)

    C2, pad_off, idx_packed, dest_cols, val_cols, counts = _preprocess(
        cfg, edge_row, edge_col, edge_val)
    nc = _build(cfg, C2, pad_off)
    in_maps = _make_in_maps(cfg, x, weight, learnable_diag, bias,
                            idx_packed, dest_cols, val_cols)
    # per-core per-gather-call true index counts (for pad-skipping)
    nonempty = (C2 > 0).reshape(-1)
    for c in range(cfg.NC):
        cc = counts[c][nonempty].astype(np.int32)
        in_maps[c]["cnt"] = cc.reshape(1, -1) if len(cc) else np.zeros(
            (1, 1), np.int32)

    kwargs = {}
    if trace_dir:
        kwargs = dict(trace=True, tmpdir=trace_dir)
    res = run_bass_kernel_spmd(nc, in_maps, core_ids=list(range(cfg.NC)),
                               **kwargs)
    out = np.empty((cfg.N, cfg.F), dtype=np.float32)
    for c in range(cfg.NC):
        out[c * cfg.RPC:(c + 1) * cfg.RPC] = res.results[c]["out"][:cfg.RPC]
    return out, res


def kernel(x, edge_row, edge_col, edge_val, weight, learnable_diag, bias,
           _want_trace=None):
    cfg = Cfg()
    out, res = run(cfg, x, edge_row, edge_col, edge_val, weight,
                   learnable_diag, bias, trace_dir=_want_trace)
    kernel._last_results = res
    return out
